# revision 56
# baseline (speedup 1.0000x reference)
"""GlobalPointer-style head (RoPE'd QK^T with pad + strict-lower-tri masks).

Self-contained Trainium2 Bass kernel. Accepts FULL inputs, shards batch 16 ->
8 cores (2 per core), runs one SPMD Bass program, gathers FULL output.

Fast path (attention_mask all ones — the harness case), ~109us/core, ~2.07x
over the 226us predecessor. The kernel is HBM-write-bound (24.6MB fp32 out
per core); the structure keeps the output-DMA stream dense from ~33us on:

  per (b, chunk c): x chunk loads race on the scalar/gpsimd DMA queues (c0
  first), fp32->bf16 cast on ACT with a (h,qk,d)->(qk,h,d) feature permute,
  RoPE on DVE in bf16 (q scaled 1/8 via tables; compact tables DMA'd then
  head-replicated on-chip). The permute makes each head PAIR's q (and k)
  features one contiguous 128-col block.
  per (b, head pair j): 8 paired transposes -> one PSUM tile (partitions
  0:64 = head 2j, 64:128 = head 2j+1), evicted DVE/ACT; per head, row-chunk
  c is one live-suffix matmul psm[:,0:512-c*128] = qc^T.T @ k^T[:,c*128:]
  (head 2j at PE base partition 0, head 2j+1 at base 64); eviction fuses the
  strict-lower-tri -NEG/8 diagonal block as a DVE tensor_add from tdiag,
  the rest is an ACT copy. The fully-masked constant prefix columns of each
  osb buffer are memset(-V8) once (they persist across pool rotations, and
  fp32 absorption makes masked entries exactly -NEG/8). One [128 x 8KB]
  output DMA per (b, h), alternating the sync/gpsimd queues; batch b+1's
  load/cast/RoPE is spread across b's head pairs so DVE never bursts.
General path (arbitrary mask): the original kernel (pad folded into k, column
bias via K=1 accumulate matmul, masked tiles copied from a broadcast row),
compiled lazily only if a non-trivial mask ever shows up.
"""

import sys

import numpy as np

for _p in ("/opt/trn_rl_repo",):
    if _p not in sys.path:
        sys.path.insert(0, _p)

import ml_dtypes  # noqa: E402

import concourse.mybir as mybir  # noqa: E402
import concourse.tile as tile  # noqa: E402
from concourse import bacc  # noqa: E402
from concourse.bass_utils import run_bass_kernel_spmd  # noqa: E402
from concourse.masks import make_identity  # noqa: E402

F32 = mybir.dt.float32
BF16 = mybir.dt.bfloat16

N_CORES = 8
B, M, H, D = 16, 512, 12, 64
BS = B // N_CORES  # batches per core
MC = M // 128  # m-chunks of 128
NEG = np.float32(1.0e12)
V8 = np.float32(NEG / np.float32(8.0))  # 1.25e11, exact in fp32


def _tables():
    """Host-precomputed constants (functions of position only, not of data)."""
    pos = np.arange(M, dtype=np.float32)[:, None]
    inv_freq = np.power(
        np.float32(10000.0),
        (np.float32(-2.0) * np.arange(D // 2, dtype=np.float32) / np.float32(D)),
    )
    ang = pos * inv_freq[None, :]  # (M, 32)
    cos = np.repeat(np.cos(ang), 2, axis=1)  # (M, 64)
    sin = np.repeat(np.sin(ang), 2, axis=1)  # (M, 64)
    sign = np.where(np.arange(D) % 2 == 0, np.float32(-1.0), np.float32(1.0))
    sin_signed = sin * sign[None, :]

    scale = np.float32(1.0 / 8.0)
    cos_t = np.concatenate([cos * scale, cos], axis=1).astype(np.float32)
    sin_t = np.concatenate([sin_signed * scale, sin_signed], axis=1).astype(np.float32)
    cos_b = cos_t.astype(ml_dtypes.bfloat16)
    sin_b = sin_t.astype(ml_dtypes.bfloat16)

    # strict-lower 128x128 diagonal block, fp32
    p = np.arange(128)
    tdiag = np.where(p[:, None] > p[None, :], -V8, np.float32(0.0)).astype(np.float32)

    # U^T @ W == tdiag (up to bf16 rounding of V8): U strict-upper ones,
    # W = diag(-V8)
    return cos_b, sin_b, tdiag


def _rope(nc, rpool, cos_sb, sin_sb, src, c):
    """bf16 RoPE on DVE for one [128, H*2*D] chunk; returns the rotated tile."""
    t1 = rpool.tile([128, H * 2 * D], BF16, tag="t1", bufs=2)
    t2 = rpool.tile([128, H * 2 * D], BF16, tag="t2", bufs=2)
    xr_c = rpool.tile([128, H * 2 * D], BF16, tag="xr", bufs=8)
    src4 = src[:].rearrange("p (h a two) -> p h a two", two=2, a=D)
    swp4 = src4[:, :, :, ::-1]
    t14 = t1[:].rearrange("p (h a two) -> p h a two", two=2, a=D)
    t24 = t2[:].rearrange("p (h a two) -> p h a two", two=2, a=D)
    xr4 = xr_c[:].rearrange("p (h a two) -> p h a two", two=2, a=D)
    cs = slice(c * 2 * D, (c + 1) * 2 * D)
    cos_b = (
        cos_sb[:, cs]
        .rearrange("p (o a two) -> p o a two", o=1, two=2)
        .to_broadcast((128, H, D, 2))
    )
    sin_b = (
        sin_sb[:, cs]
        .rearrange("p (o a two) -> p o a two", o=1, two=2)
        .to_broadcast((128, H, D, 2))
    )
    nc.vector.tensor_mul(out=t14, in0=swp4, in1=sin_b)
    nc.vector.tensor_mul(out=t24, in0=src4, in1=cos_b)
    nc.vector.tensor_add(out=xr4, in0=t24, in1=t14)
    return xr_c


def build_nc_fast():
    """Fast path: attention_mask is all ones (no pad masking).

    x features are permuted to (qk, h, d) at the cast so q/k of a head PAIR
    are contiguous 128-col blocks for the transposes; RoPE uses
    head-replicated tables in the same layout.
    """
    nc = bacc.Bacc("TRN2", target_bir_lowering=False, debug=False)

    F = H * 2 * D  # 1536 features
    x_d = nc.dram_tensor("x", [BS, M, F], F32, kind="ExternalInput")
    cos_d = nc.dram_tensor("cos_t", [M, 2 * D], BF16, kind="ExternalInput")
    sin_d = nc.dram_tensor("sin_t", [M, 2 * D], BF16, kind="ExternalInput")
    tdiag_d = nc.dram_tensor("tdiag", [128, 128], F32, kind="ExternalInput")
    out_d = nc.dram_tensor("out", [BS, H, M, M], F32, kind="ExternalOutput")

    def pairs(t):  # (g=qk*h, a=sin/cos pair, two) view for the rotate-half
        return t.rearrange("p (g a two) -> p g a two", a=D // 2, two=2)

    with tile.TileContext(nc) as tc:
        with (
            tc.tile_pool(name="const", bufs=1) as cpool,
            tc.tile_pool(name="xin", bufs=2) as xpool,
            tc.tile_pool(name="rope", bufs=2) as rpool,
            tc.tile_pool(name="qkt", bufs=3) as kpool,
            tc.tile_pool(name="osb", bufs=6) as opool,
            tc.tile_pool(name="ps_t", bufs=2, space="PSUM") as pst_pool,
            tc.tile_pool(name="ps_mm", bufs=6, space="PSUM") as psm_pool,
        ):
            # compact [M, 128] tables from DRAM first, on the scalar queue
            # AHEAD of the x loads so they land early, then head-replicate
            # on-chip lazily per chunk
            cos_c = cpool.tile([128, MC * 2 * D], BF16)
            nc.scalar.dma_start(
                out=cos_c[:].rearrange("p (c f) -> p c f", c=MC),
                in_=cos_d[:].rearrange("(c p) f -> p c f", p=128),
            )
            sin_c = cpool.tile([128, MC * 2 * D], BF16)
            nc.scalar.dma_start(
                out=sin_c[:].rearrange("p (c f) -> p c f", c=MC),
                in_=sin_d[:].rearrange("(c p) f -> p c f", p=128),
            )
            ident = cpool.tile([128, 128], BF16)
            make_identity(nc, ident)
            tdiag_sb = cpool.tile([128, 128], F32)
            nc.sync.dma_start(out=tdiag_sb[:], in_=tdiag_d[:])

            cos_sb = cpool.tile([128, MC * F], BF16)
            sin_sb = cpool.tile([128, MC * F], BF16)

            def replicate_tables(c):
                eng = nc.vector
                for qk in range(2):
                    for big, small in ((cos_sb, cos_c), (sin_sb, sin_c)):
                        src = (
                            small[:, c * 2 * D + qk * D : c * 2 * D + (qk + 1) * D]
                            .rearrange("p (o d) -> p o d", o=1)
                            .to_broadcast((128, H, D))
                        )
                        dst = big[
                            :, c * F + qk * H * D : c * F + (qk + 1) * H * D
                        ].rearrange("p (h d) -> p h d", d=D)
                        eng.tensor_copy(out=dst, in_=src)

            def load_chunk(b, c):
                # batch 0: scalar gets (c0, c2), gpsimd gets (c1, c3) so
                # c0/c1 land first in parallel and the pipeline starts early.
                # later batches: scalar only — the gpsimd queue carries output
                # DMAs by then and a 1.5MB x transfer would block them.
                t = xpool.tile([128, F], F32, tag="xn", bufs=4)
                qeng = nc.scalar if (b > 0 or c % 2 == 0) else nc.gpsimd
                qeng.dma_start(out=t[:], in_=x_d[b, c * 128 : (c + 1) * 128, :])
                return t

            def prep_chunk(b, c, t):
                # cast to bf16 on ACT with the (h, qk, d) -> (qk, h, d)
                # feature permutation, then RoPE on DVE (q scaled 1/8)
                tb = xpool.tile([128, F], BF16, tag="xb", bufs=8)
                nc.scalar.copy(
                    out=tb[:].rearrange("p (qk h d) -> p h qk d", qk=2, d=D),
                    in_=t[:].rearrange("p (h qk d) -> p h qk d", qk=2, d=D),
                )
                if b == 0:
                    replicate_tables(c)
                cs = slice(c * F, (c + 1) * F)
                t1 = rpool.tile([128, F], BF16, tag="t1", bufs=2)
                t2 = rpool.tile([128, F], BF16, tag="t2", bufs=2)
                xr_c = rpool.tile([128, F], BF16, tag="xr", bufs=8)
                nc.vector.tensor_mul(
                    out=pairs(t1[:]),
                    in0=pairs(tb[:])[:, :, :, ::-1],
                    in1=pairs(sin_sb[:, cs]),
                )
                nc.vector.tensor_mul(out=t2[:], in0=tb[:], in1=cos_sb[:, cs])
                nc.vector.tensor_add(out=xr_c[:], in0=t2[:], in1=t1[:])
                return xr_c

            bh_idx = 0
            xn1 = None
            xr_next = []
            for b in range(BS):
                if b == 0:
                    xn0 = [load_chunk(0, c) for c in range(MC)]
                    xr = [prep_chunk(0, c, xn0[c]) for c in range(MC)]
                else:
                    xr = xr_next

                for j in range(H // 2):  # head pairs (2j, 2j+1)
                    # spread the next batch's load/cast/RoPE across this
                    # batch's head pairs so the DVE never sees a burst
                    if b + 1 < BS:
                        if j == 1:
                            xn1 = [load_chunk(b + 1, c) for c in range(MC)]
                        elif 2 <= j < 2 + MC:
                            xr_next.append(prep_chunk(b + 1, j - 2, xn1[j - 2]))
                    # paired transposes: q feats of both heads -> one
                    # [128, 128] block (partitions 0:64 = head 2j's q^T,
                    # 64:128 = head 2j+1's); same for k. head 2j matmuls at
                    # PE base partition 0, head 2j+1 at base 64.
                    pst = pst_pool.tile([128, 2 * MC * 128], BF16, tag="pst")
                    qo = 2 * j * D  # q block offset in (qk, h, d) layout
                    ko = (H + 2 * j) * D
                    for c in range(MC):
                        nc.tensor.transpose(
                            pst[:, c * 128 : (c + 1) * 128],
                            xr[c][:, qo : qo + 128],
                            ident[:],
                        )
                        nc.tensor.transpose(
                            pst[:, M + c * 128 : M + (c + 1) * 128],
                            xr[c][:, ko : ko + 128],
                            ident[:],
                        )
                    qkt = kpool.tile([128, 2 * MC * 128], BF16, tag="qkt")
                    nc.vector.tensor_copy(out=qkt[:], in_=pst[:])

                    for hh in range(2):
                        pb = 64 * hh  # PE base partition for this head
                        osb = opool.tile([128, MC * M], F32, tag="osb")
                        if bh_idx < 6:
                            # fully-masked constant prefix: persists across
                            # pool buffer rotations, so only the first 6
                            # iterations (one per buffer) write it
                            for c in range(1, MC):
                                nc.gpsimd.memset(
                                    osb[:, c * M : c * M + c * 128], float(-V8)
                                )
                        for c in range(MC):
                            live = M - c * 128
                            psm = psm_pool.tile([128, M], F32, tag="psmm")
                            nc.tensor.matmul(
                                psm[:, 0:live],
                                qkt[pb : pb + 64, c * 128 : (c + 1) * 128],
                                qkt[pb : pb + 64, M + c * 128 : 2 * M],
                                start=True,
                                stop=True,
                            )
                            # diag block: strict-lower-tri -V8 add fused into
                            # the PSUM eviction (DVE/GPSIMD); rest ACT copy
                            base = c * M + c * 128
                            nc.vector.tensor_add(
                                out=osb[:, base : base + 128],
                                in0=psm[:, 0:128],
                                in1=tdiag_sb[:],
                            )
                            if live > 128:
                                nc.scalar.copy(
                                    out=osb[:, base + 128 : (c + 1) * M],
                                    in_=psm[:, 128:live],
                                )
                        oeng = nc.sync if bh_idx % 2 == 0 else nc.gpsimd
                        oeng.dma_start(
                            out=out_d[b, 2 * j + hh].rearrange(
                                "(c p) n -> p c n", p=128
                            ),
                            in_=osb[:].rearrange("p (c n) -> p c n", c=MC),
                        )
                        bh_idx += 1

    nc.compile()
    return nc


def build_nc_general():
    nc = bacc.Bacc("TRN2", target_bir_lowering=False, debug=False)

    x_d = nc.dram_tensor("x", [BS, M, H * 2 * D], F32, kind="ExternalInput")
    mask_d = nc.dram_tensor("mask", [BS, M], F32, kind="ExternalInput")
    cos_d = nc.dram_tensor("cos_t", [M, 2 * D], BF16, kind="ExternalInput")
    sin_d = nc.dram_tensor("sin_t", [M, 2 * D], BF16, kind="ExternalInput")
    tdiag_d = nc.dram_tensor("tdiag", [128, 128], F32, kind="ExternalInput")
    out_d = nc.dram_tensor("out", [BS, H, M, M], F32, kind="ExternalOutput")

    mult = mybir.AluOpType.mult
    add = mybir.AluOpType.add

    with tile.TileContext(nc) as tc:
        with (
            tc.tile_pool(name="const", bufs=1) as cpool,
            tc.tile_pool(name="xin", bufs=2) as xpool,
            tc.tile_pool(name="rope", bufs=2) as rpool,
            tc.tile_pool(name="small", bufs=2) as spool,
            tc.tile_pool(name="xt", bufs=3) as tpool,
            tc.tile_pool(name="osb", bufs=3) as opool,
            tc.tile_pool(name="ps_t", bufs=3, space="PSUM") as pst_pool,
            tc.tile_pool(name="ps_mm", bufs=2, space="PSUM") as psm_pool,
        ):
            ident = cpool.tile([128, 128], BF16)
            make_identity(nc, ident)
            ones_row = cpool.tile([1, 128], BF16)
            nc.gpsimd.memset(ones_row[:], 1.0)
            ones_f32 = cpool.tile([1, 128], F32)
            nc.gpsimd.memset(ones_f32[:], 1.0)

            cos_sb = cpool.tile([128, MC * 2 * D], BF16)
            nc.sync.dma_start(
                out=cos_sb[:].rearrange("p (c f) -> p c f", c=MC),
                in_=cos_d[:].rearrange("(c p) f -> p c f", p=128),
            )
            sin_sb = cpool.tile([128, MC * 2 * D], BF16)
            nc.sync.dma_start(
                out=sin_sb[:].rearrange("p (c f) -> p c f", c=MC),
                in_=sin_d[:].rearrange("(c p) f -> p c f", p=128),
            )
            tdiag_sb = cpool.tile([128, 128], F32)
            nc.sync.dma_start(out=tdiag_sb[:], in_=tdiag_d[:])

            copy_rr = 0  # round-robin the PSUM->SBUF copies across ACT/DVE
            for b in range(BS):
                # ---- load x[b] fp32, cast to bf16 on GPSIMD
                xb = []
                for c in range(MC):
                    t = xpool.tile([128, H * 2 * D], F32, tag="xn", bufs=4)
                    nc.sync.dma_start(out=t[:], in_=x_d[b, c * 128 : (c + 1) * 128, :])
                    tb = xpool.tile([128, H * 2 * D], BF16, tag="xb", bufs=8)
                    nc.gpsimd.tensor_copy(out=tb[:], in_=t[:])
                    xb.append(tb)

                # ---- colbias rows: colstep + padbias (broadcast 4x), GPSIMD
                padrow = spool.tile([1, M], F32, tag="padrow")
                nc.sync.dma_start(out=padrow[:], in_=mask_d[b : b + 1, :])
                padbias = spool.tile([1, M], F32, tag="padbias")
                nc.vector.tensor_scalar(
                    out=padbias[:],
                    in0=padrow[:],
                    scalar1=float(V8),
                    scalar2=float(-V8),
                    op0=mult,
                    op1=add,
                )
                colb = spool.tile([1, M], BF16, tag="colb")
                nc.gpsimd.tensor_copy(out=colb[:], in_=padbias[:])
                # exact fp32 value of fully-masked columns: -V8*(2-pad[n]),
                # replicated to all partitions by a K=1 matmul
                rowvals = spool.tile([1, M], F32, tag="rowvals")
                nc.vector.tensor_scalar(
                    out=rowvals[:],
                    in0=padrow[:],
                    scalar1=float(V8),
                    scalar2=float(-2.0 * V8),
                    op0=mult,
                    op1=add,
                )
                ps_cf = pst_pool.tile([128, M], F32, tag="pscf", bufs=1)
                nc.tensor.matmul(
                    ps_cf[:], ones_f32[:], rowvals[:], start=True, stop=True
                )
                colfull = spool.tile([128, M], F32, tag="colfull")
                nc.vector.tensor_copy(out=colfull[:], in_=ps_cf[:])

                # ---- pad as per-partition column per m-chunk (k scaling)
                padcol = spool.tile([128, MC], F32, tag="padcol")
                nc.sync.dma_start(
                    out=padcol[:], in_=mask_d[b, :].rearrange("(c p) -> p c", p=128)
                )

                # ---- RoPE in bf16, q scaled 1/8, k scaled by pad
                xr = []
                for c in range(MC):
                    xr_c = _rope(nc, rpool, cos_sb, sin_sb, xb[c], c)
                    k3 = xr_c[:].rearrange("p (h f) -> p h f", f=2 * D)[:, :, D:]
                    nc.vector.tensor_scalar(
                        out=k3,
                        in0=k3,
                        scalar1=padcol[:, c : c + 1],
                        scalar2=None,
                        op0=mult,
                    )
                    xr.append(xr_c)

                # ---- per head: transpose, matmul (+K=1 colbias), epilogue
                for h in range(H):
                    qkt = tpool.tile([D, MC * 2 * 128], BF16, tag="qkt")
                    for c in range(MC):
                        ps_t = pst_pool.tile([D, 256], BF16, tag="pst", bufs=3)
                        nc.tensor.transpose(
                            ps_t[:, 0:128],
                            xr[c][:, h * 2 * D : h * 2 * D + D],
                            ident[:],
                        )
                        nc.tensor.transpose(
                            ps_t[:, 128:256],
                            xr[c][:, h * 2 * D + D : (h + 1) * 2 * D],
                            ident[:],
                        )
                        nc.scalar.copy(
                            out=qkt[:, c * 256 : (c + 1) * 256], in_=ps_t[:]
                        )
                    qkt3 = qkt[:].rearrange("p (c two f) -> p c two f", two=2, f=128)
                    kt_ap = qkt3[:, :, 1, :]  # (D, MC, 128) strided k view
                    osb = opool.tile([128, MC * M], F32, tag="osb")
                    for g in range(2):
                        ps_mm = psm_pool.tile([128, 2 * M], F32, tag="psmm", bufs=2)
                        for cc in range(2):
                            c = g * 2 + cc
                            nc.tensor.matmul(
                                ps_mm[:, cc * M : (cc + 1) * M],
                                qkt[:, c * 256 : c * 256 + 128],
                                kt_ap,
                                start=True,
                                stop=False,
                            )
                            # rank-1 column-bias accumulate (K=1)
                            nc.tensor.matmul(
                                ps_mm[:, cc * M : (cc + 1) * M],
                                ones_row[:],
                                colb[0:1, :],
                                start=False,
                                stop=True,
                            )
                        for cc in range(2):
                            c = g * 2 + cc
                            # left c*128 cols are exact constants; copy only
                            # the live suffix out of PSUM (ACT/DVE round
                            # robin), then in-place diag-block add on DVE
                            lw = c * 128
                            use_act = (copy_rr % 8) < 5
                            copy_rr += 1
                            if lw:
                                nc.gpsimd.tensor_copy(
                                    out=osb[:, c * M : c * M + lw],
                                    in_=colfull[:, 0:lw],
                                )
                            if use_act:
                                nc.scalar.copy(
                                    out=osb[:, c * M + lw : (c + 1) * M],
                                    in_=ps_mm[:, cc * M + lw : (cc + 1) * M],
                                )
                            else:
                                nc.vector.tensor_copy(
                                    out=osb[:, c * M + lw : (c + 1) * M],
                                    in_=ps_mm[:, cc * M + lw : (cc + 1) * M],
                                )
                            ds = slice(c * M + lw, c * M + lw + 128)
                            nc.vector.tensor_add(
                                out=osb[:, ds], in0=osb[:, ds], in1=tdiag_sb[:]
                            )
                    nc.sync.dma_start(
                        out=out_d[b, h].rearrange("(c p) n -> p c n", p=128),
                        in_=osb[:].rearrange("p (c n) -> p c n", c=MC),
                    )

    nc.compile()
    return nc


_NC_FAST = None
_NC_GEN = None
_TABLES = None


def _get_tables():
    global _TABLES
    if _TABLES is None:
        _TABLES = _tables()
    return _TABLES


def _get_nc(fast):
    global _NC_FAST, _NC_GEN
    if fast:
        if _NC_FAST is None:
            _NC_FAST = build_nc_fast()
        return _NC_FAST
    if _NC_GEN is None:
        _NC_GEN = build_nc_general()
    return _NC_GEN


def run(x, attention_mask, **run_kwargs):
    cos_b, sin_b, tdiag = _get_tables()
    x = np.ascontiguousarray(np.asarray(x, dtype=np.float32))
    am = np.ascontiguousarray(np.asarray(attention_mask, dtype=np.float32))
    fast = bool(np.all(am == np.float32(1.0)))
    nc = _get_nc(fast)
    maps = []
    for i in range(N_CORES):
        sl = slice(i * BS, (i + 1) * BS)
        m = {
            "x": np.ascontiguousarray(x[sl]),
            "cos_t": cos_b,
            "sin_t": sin_b,
            "tdiag": tdiag,
        }
        if not fast:
            m["mask"] = np.ascontiguousarray(am[sl])
        maps.append(m)
    res = run_bass_kernel_spmd(nc, maps, list(range(N_CORES)), **run_kwargs)
    out = np.concatenate([r["out"] for r in res.results], axis=0)
    return out, res


def kernel(x, attention_mask, token_type_ids=None, **_unused):
    out, _ = run(x, attention_mask)
    return out
